# revision 11
# baseline (speedup 1.0000x reference)
"""Expert-LoRA routed delta kernel for Trainium2 (8 NeuronCores).

Math (per batch b, with routing resolved on host):
    out[b] = base[b] + x[b] @ At_b @ Bwt_b
where
    At_b  [H, 32] = concat_k A_{e_k}^T              (e_k = top_k_indices[b, k])
    Bwt_b [32, H] = concat_k (w_{b,k} * scaling * B_{e_k}^T)

The kernel is pure HBM-bandwidth-bound (rank-32 LoRA => tiny FLOPs vs 3
full passes over [S, H]), so all device I/O is bf16: host-side prep casts
x/base and the tiny At/Bwt tables to bf16 (max rel err ~5e-3, well inside
the 2e-2 gate), halving HBM traffic vs fp32 — 44 MB/core instead of 88.

Host-side prep folds everything cheap into input layout:
  * expert gather + gate weights + lora scaling -> tiny At/Bwt tables;
  * x is pre-transposed to an h-major tiled layout xt[half, p, j, s] so the
    tensor engine contracts over H without on-chip transposes AND each DMA
    is one fully contiguous 1.84 MB block (14 KB per partition).

Device pipeline per core (= one batch; B == n_cores == 8):
  for each 512-row S-macro: load xT halves -> 28 accumulating matmuls
  (rank-32 down-projection, N=512) -> per 128-row block: 7 up-projection
  matmuls (K=32, N=512) -> drain each PSUM result (add base, cast bf16)
  via one of two engine paths balanced so PE/DVE/ACT all sit near their
  common ~48 us floor (see build_nc docstring) -> store on the gpsimd
  SWDGE ring. bf16 matmuls run 1 output column/cycle, so the mandatory
  PE work (down-proj streams all of x once; up-proj writes every output
  once) is 2 x 24 us/core — the binding constraint alongside the drain.

Sharding: data-parallel over batch (spec sharding_hint), SPMD program.
"""

import sys

if "/opt/trn_rl_repo" not in sys.path:
    sys.path.insert(0, "/opt/trn_rl_repo")

import numpy as np
import ml_dtypes

BF16 = ml_dtypes.bfloat16

# Problem shape (hardcoded per contract; must match setup_inputs()).
B, S, H = 8, 2048, 3584
E, R, TOPK = 8, 16, 2
KR = TOPK * R  # 32 = concatenated rank
SCALING = 32.0 / 16.0
N_CORES = 8

S_BLK = 128
HB = H // 128  # 28 h-blocks of 128
HC = H // 512  # 7 h-chunks of 512
NMAC = S // 512  # 4 S-macros of 512 rows
HHALF = HB // 2  # 14 h-blocks per xT half-tile

_CACHE: dict = {}


def _split_sync_waits(nc, max_waits=1):
    """This walrus build rejects >max_waits sync-wait commands on a single
    instruction (setupSyncWait: 'Too many sync wait commands'). Hoist excess
    waits onto same-engine NOPs inserted immediately before the instruction.
    Same-queue ordering makes this equivalent: the engine blocks on each
    hoisted wait before reaching the original instruction. Monotonic (ge)
    waits are hoisted first; eq-waits stay on the instruction when possible.
    """
    import concourse.mybir as mybir

    for fn in nc.m.functions:
        for bb in fn.blocks:
            new_insts = []
            for inst in bb.instructions:
                si = inst.sync_info
                if si is not None and si.on_wait and len(si.on_wait) > max_waits:
                    waits = list(si.on_wait)
                    ge = [w for w in waits if w.wait_mode != "sem-eq-imm"]
                    eq = [w for w in waits if w.wait_mode == "sem-eq-imm"]
                    keep = (eq + ge)[-max_waits:]
                    hoist = (eq + ge)[:-max_waits]
                    for w in hoist:
                        new_insts.append(
                            mybir.InstNoOp(
                                name=f"I-{nc.next_id()}",
                                engine=inst.engine,
                                bass_nofuse=True,
                                sync_info=mybir.SyncInfo(on_wait=[w], on_update=[]),
                            )
                        )
                    inst.sync_info = mybir.SyncInfo(
                        on_wait=keep, on_update=list(si.on_update or [])
                    )
                new_insts.append(inst)
            bb.instructions[:] = new_insts


def _spread(k, n, total):
    """Bresenham spread: True for k such that exactly n of `total` fire."""
    return (k * n) // total != ((k + 1) * n) // total


def build_nc(reps=1, dma_only=False, io_bufs=4, xt_bufs=6,
             nv_super=16, nv_single=4, base_eng="sync", store_eng="gpsimd",
             plow_bufs=2, pd2_bufs=2, pd1_bufs=2):
    """Build the single-core Bass program (SPMD: same program on all cores).

    reps>1 repeats the whole pipeline (same I/O, idempotent) — used only for
    slope-based device-time measurement in test.py. dma_only strips compute
    (out <- base, xT still loaded) to calibrate the pure DMA roofline.

    Drain of each up-projection PSUM result (add base + cast to bf16) goes
    down one of two paths, balanced so DVE/ACT/PE all sit near the same
    busy time:
      V: DVE tensor_add(psum f32 + base bf16 -> out bf16), 1 elem/cycle;
      C: ACT copy psum->out bf16, then DVE in-place 2x-mode bf16 add of base.
    Chunks are paired into [128, 1024] PSUM super-tiles (2 banks) to halve
    per-instruction overheads; nv_super/nv_single set how many of the 48
    super / 16 single drains per program take the V path.
    base_eng/store_eng pick the DMA ring for base loads / out stores
    (sync=SP HWDGE, scalar=ACT HWDGE, gpsimd=SWDGE). Stores default to the
    otherwise-idle gpsimd ring so a store waiting on a drain never
    head-of-line-blocks ACT's copies or SP's loads.
    """
    import concourse.bass as bass
    import concourse.mybir as mybir
    import concourse.tile as tile

    bf16 = mybir.dt.bfloat16
    f32 = mybir.dt.float32
    Copy = mybir.ActivationFunctionType.Copy
    nc = bass.Bass()
    # xt[half, p, j, s] = x[(half//2)*512 + s, (half%2)*1792 + j*128 + p]
    xt = nc.dram_tensor("xt", [2 * NMAC, 128, HHALF, 512], bf16, kind="ExternalInput")
    base = nc.dram_tensor("base", [S, H], bf16, kind="ExternalInput")
    # at[p, j, r] = A_cat^T[j*128 + p, r] (pre-striped on host)
    at = nc.dram_tensor("at", [128, HB, KR], bf16, kind="ExternalInput")
    bwt = nc.dram_tensor("bwt", [KR, H], bf16, kind="ExternalInput")
    out = nc.dram_tensor("out", [S, H], bf16, kind="ExternalOutput")

    engs = {"sync": nc.sync, "scalar": nc.scalar, "gpsimd": nc.gpsimd}
    store_eng = engs[store_eng]
    b_eng = engs[base_eng]

    with tile.TileContext(nc) as tc:
        with (
            tc.tile_pool(name="const", bufs=1) as const_pool,
            tc.tile_pool(name="xth", bufs=xt_bufs) as xt_pool,
            tc.tile_pool(name="bin", bufs=io_bufs) as b_pool,
            tc.tile_pool(name="oout", bufs=io_bufs) as o_pool,
            tc.tile_pool(name="low", bufs=3) as low_pool,
            tc.tile_pool(name="plow", bufs=plow_bufs, space="PSUM") as plow_pool,
            tc.tile_pool(name="pd2", bufs=pd2_bufs, space="PSUM") as pd2_pool,
            tc.tile_pool(name="pd1", bufs=pd1_bufs, space="PSUM") as pd1_pool,
        ):
            at_sb = const_pool.tile([128, HB, KR], bf16)
            nc.sync.dma_start(at_sb[:], at[:])
            bwt_sb = const_pool.tile([KR, H], bf16)
            nc.sync.dma_start(bwt_sb[:], bwt[:])

            n_super = 0  # of 48 per program: V-path if _spread(., nv_super, 48)
            n_single = 0  # of 16 per program
            for m in range(NMAC * reps):
                m = m % NMAC
                # xT halves: [128 h-partitions, 14 h-blocks, 512 s]
                halves = []
                for hf in range(2):
                    xh = xt_pool.tile([128, HHALF, 512], bf16, tag="xth")
                    nc.sync.dma_start(xh[:], xt[2 * m + hf])
                    halves.append(xh)

                if not dma_only:
                    # down-projection: lowT[kr, s] = sum_h At[h, kr] * xT[h, s]
                    plow = plow_pool.tile([KR, 512], f32, tag="plow")
                    for j in range(HB):
                        nc.tensor.matmul(
                            plow[:],
                            at_sb[:, j, :],
                            halves[j // HHALF][:, j % HHALF, :],
                            start=(j == 0),
                            stop=(j == HB - 1),
                        )
                    lowT = low_pool.tile([KR, 512], bf16, tag="lowT")
                    nc.scalar.activation(lowT[:], plow[:], Copy)

                for g in range(4):  # 128-row s-blocks within the macro
                    srow = m * 512 + g * S_BLK
                    bt = b_pool.tile([S_BLK, H], bf16, tag="base")
                    b_eng.dma_start(bt[:], base[srow : srow + S_BLK, :])
                    if dma_only:
                        store_eng.dma_start(out[srow : srow + S_BLK, :], bt[:])
                        continue
                    ot = o_pool.tile([S_BLK, H], bf16, tag="out")
                    lg = lowT[:, g * S_BLK : (g + 1) * S_BLK]
                    # 3 paired chunks of 1024 + 1 single of 512 (H = 3584)
                    for ci, (c0, width) in enumerate(
                        [(0, 1024), (1024, 1024), (2048, 1024), (3072, 512)]
                    ):
                        if width == 1024:
                            pd = pd2_pool.tile([S_BLK, 1024], f32, tag="pd2")
                            v_path = _spread(n_super % 48, nv_super, 48)
                            n_super += 1
                        else:
                            pd = pd1_pool.tile([S_BLK, 512], f32, tag="pd1")
                            v_path = _spread(n_single % 16, nv_single, 16)
                            n_single += 1
                        for o in range(0, width, 512):
                            nc.tensor.matmul(
                                pd[:, o : o + 512],
                                lg,
                                bwt_sb[:, c0 + o : c0 + o + 512],
                                start=True,
                                stop=True,
                            )
                        osl = ot[:, c0 : c0 + width]
                        bsl = bt[:, c0 : c0 + width]
                        if v_path:
                            # V: single DVE pass, PSUM operand (1 elem/cyc)
                            nc.vector.tensor_add(osl, pd[:], bsl)
                        else:
                            # C: ACT drains PSUM, DVE adds base in 2x mode
                            nc.scalar.activation(osl, pd[:], Copy)
                            nc.vector.tensor_add(osl, osl, bsl)
                    store_eng.dma_start(out[srow : srow + S_BLK, :], ot[:])

    _split_sync_waits(nc)
    return nc


def make_in_maps(x, base_output, lora_A, lora_B, top_k_weights, top_k_indices):
    """Host-side prep: expert gather, gate/scaling fold, bf16 cast, x h-major
    relayout so every device DMA is large and fully contiguous."""
    x = np.asarray(x, dtype=np.float32)
    base_output = np.asarray(base_output, dtype=np.float32)
    lora_A = np.asarray(lora_A, dtype=np.float32)
    lora_B = np.asarray(lora_B, dtype=np.float32)
    w = np.asarray(top_k_weights, dtype=np.float32)
    idx = np.asarray(top_k_indices)

    A_sel = lora_A[idx]  # [B, K, R, H]
    At = A_sel.reshape(B, KR, H)  # [B, 32, H] (row r = A_cat[r, :])
    # stripe h-major: at[b, p, j, r] = A_cat[b, r, j*128 + p]
    At_dev = np.ascontiguousarray(
        At.reshape(B, KR, HB, 128).transpose(0, 3, 2, 1)
    ).astype(BF16)  # [B, 128, 28, 32]
    B_sel = lora_B[idx]  # [B, K, H, R]
    Bw = B_sel * (w * SCALING)[:, :, None, None]
    Bwt = np.ascontiguousarray(
        Bw.transpose(0, 1, 3, 2).reshape(B, KR, H)
    ).astype(BF16)  # [B, 32, H]

    # x -> xt[half, p, j, s]: h-major tiles, fully contiguous per half
    # xt[b, 2m+hf, p, j, s] = x[b, m*512 + s, hf*1792 + j*128 + p]
    xb = x.astype(BF16)
    xt = np.ascontiguousarray(
        xb.reshape(B, NMAC, 512, 2, HHALF, 128)
        .transpose(0, 1, 3, 5, 4, 2)  # [B, m, hf, p, j, s]
        .reshape(B, 2 * NMAC, 128, HHALF, 512)
    )

    return [
        {
            "xt": xt[b],
            "base": np.ascontiguousarray(base_output[b]).astype(BF16),
            "at": At_dev[b],
            "bwt": Bwt[b],
        }
        for b in range(B)
    ]


def kernel(x, base_output, lora_A, lora_B, top_k_weights, top_k_indices):
    from concourse.bass_utils import run_bass_kernel_spmd

    nc = _CACHE.get("nc")
    if nc is None:
        nc = build_nc()
        _CACHE["nc"] = nc

    in_maps = make_in_maps(
        x, base_output, lora_A, lora_B, top_k_weights, top_k_indices
    )
    res = run_bass_kernel_spmd(nc, in_maps, list(range(N_CORES)))
    return np.stack(
        [res.results[b]["out"].astype(np.float32) for b in range(B)], axis=0
    )


# revision 14
# speedup vs baseline: 2.8505x; 2.8505x over previous
"""Expert-LoRA routed delta kernel for Trainium2 (8 NeuronCores).

Math (per batch b, with routing resolved on host):
    out[b] = base[b] + x[b] @ At_b @ Bwt_b
where
    At_b  [H, 32] = concat_k A_{e_k}^T              (e_k = top_k_indices[b, k])
    Bwt_b [32, H] = concat_k (w_{b,k} * scaling * B_{e_k}^T)

The kernel is pure HBM-bandwidth-bound (rank-32 LoRA => tiny FLOPs vs 3
full passes over [S, H]), so all device I/O is bf16: host-side prep casts
x/base and the tiny At/Bwt tables to bf16 (max rel err ~5e-3, well inside
the 2e-2 gate), halving HBM traffic vs fp32 — 44 MB/core instead of 88.

Host-side prep folds everything cheap into input layout:
  * expert gather + gate weights + lora scaling -> tiny At/Bwt tables;
  * x is pre-transposed to an h-major tiled layout xt[half, p, j, s] so the
    tensor engine contracts over H without on-chip transposes AND each DMA
    is one fully contiguous 1.84 MB block (14 KB per partition).

Device pipeline per core (= one batch; B == n_cores == 8):
  for each 512-row S-macro: load xT halves -> 28 accumulating matmuls
  (rank-32 down-projection, N=512) -> per 128-row block: 7 up-projection
  matmuls (K=32, N=512) -> drain each PSUM result (add base, cast bf16)
  via one of two engine paths balanced so PE/DVE/ACT all sit near their
  common ~48 us floor (see build_nc docstring) -> store on the gpsimd
  SWDGE ring. bf16 matmuls run 1 output column/cycle, so the mandatory
  PE work (down-proj streams all of x once; up-proj writes every output
  once) is 2 x 24 us/core — the binding constraint alongside the drain.

Sharding: data-parallel over batch (spec sharding_hint), SPMD program.
"""

import sys

if "/opt/trn_rl_repo" not in sys.path:
    sys.path.insert(0, "/opt/trn_rl_repo")

import numpy as np
import ml_dtypes

BF16 = ml_dtypes.bfloat16

# Problem shape (hardcoded per contract; must match setup_inputs()).
B, S, H = 8, 2048, 3584
E, R, TOPK = 8, 16, 2
KR = TOPK * R  # 32 = concatenated rank
SCALING = 32.0 / 16.0
N_CORES = 8

S_BLK = 128
HB = H // 128  # 28 h-blocks of 128
HC = H // 512  # 7 h-chunks of 512
NMAC = S // 512  # 4 S-macros of 512 rows
HHALF = HB // 2  # 14 h-blocks per xT half-tile

_CACHE: dict = {}


def _split_sync_waits(nc, max_waits=1):
    """This walrus build rejects >max_waits sync-wait commands on a single
    instruction (setupSyncWait: 'Too many sync wait commands'). Hoist excess
    waits onto same-engine NOPs inserted immediately before the instruction.
    Same-queue ordering makes this equivalent: the engine blocks on each
    hoisted wait before reaching the original instruction. Monotonic (ge)
    waits are hoisted first; eq-waits stay on the instruction when possible.
    """
    import concourse.mybir as mybir

    for fn in nc.m.functions:
        for bb in fn.blocks:
            new_insts = []
            for inst in bb.instructions:
                si = inst.sync_info
                if si is not None and si.on_wait and len(si.on_wait) > max_waits:
                    waits = list(si.on_wait)
                    ge = [w for w in waits if w.wait_mode != "sem-eq-imm"]
                    eq = [w for w in waits if w.wait_mode == "sem-eq-imm"]
                    keep = (eq + ge)[-max_waits:]
                    hoist = (eq + ge)[:-max_waits]
                    for w in hoist:
                        new_insts.append(
                            mybir.InstNoOp(
                                name=f"I-{nc.next_id()}",
                                engine=inst.engine,
                                bass_nofuse=True,
                                sync_info=mybir.SyncInfo(on_wait=[w], on_update=[]),
                            )
                        )
                    inst.sync_info = mybir.SyncInfo(
                        on_wait=keep, on_update=list(si.on_update or [])
                    )
                new_insts.append(inst)
            bb.instructions[:] = new_insts


def _spread(k, n, total):
    """Bresenham spread: True for k such that exactly n of `total` fire."""
    return (k * n) // total != ((k + 1) * n) // total


def build_nc(reps=1, dma_only=False, io_bufs=4, xt_bufs=6,
             nv_super=16, nv_single=4, base_eng="sync", store_eng="gpsimd",
             plow_bufs=2, pd2_bufs=2, pd1_bufs=2, pipe=False):
    """Build the single-core Bass program (SPMD: same program on all cores).

    reps>1 repeats the whole pipeline (same I/O, idempotent) — used only for
    slope-based device-time measurement in test.py. dma_only strips compute
    (out <- base, xT still loaded) to calibrate the pure DMA roofline.

    Drain of each up-projection PSUM result (add base + cast to bf16) goes
    down one of two paths, balanced so DVE/ACT/PE all sit near the same
    busy time:
      V: DVE tensor_add(psum f32 + base bf16 -> out bf16), 1 elem/cycle;
      C: ACT copy psum->out bf16, then DVE in-place 2x-mode bf16 add of base.
    Chunks are paired into [128, 1024] PSUM super-tiles (2 banks) to halve
    per-instruction overheads; nv_super/nv_single set how many of the 48
    super / 16 single drains per program take the V path.
    base_eng/store_eng pick the DMA ring for base loads / out stores
    (sync=SP HWDGE, scalar=ACT HWDGE, gpsimd=SWDGE). Stores default to the
    otherwise-idle gpsimd ring so a store waiting on a drain never
    head-of-line-blocks ACT's copies or SP's loads.
    """
    import concourse.bass as bass
    import concourse.mybir as mybir
    import concourse.tile as tile

    bf16 = mybir.dt.bfloat16
    f32 = mybir.dt.float32
    Copy = mybir.ActivationFunctionType.Copy
    nc = bass.Bass()
    # xt[half, p, j, s] = x[(half//2)*512 + s, (half%2)*1792 + j*128 + p]
    xt = nc.dram_tensor("xt", [2 * NMAC, 128, HHALF, 512], bf16, kind="ExternalInput")
    base = nc.dram_tensor("base", [S, H], bf16, kind="ExternalInput")
    # at[p, j, r] = A_cat^T[j*128 + p, r] (pre-striped on host)
    at = nc.dram_tensor("at", [128, HB, KR], bf16, kind="ExternalInput")
    bwt = nc.dram_tensor("bwt", [KR, H], bf16, kind="ExternalInput")
    out = nc.dram_tensor("out", [S, H], bf16, kind="ExternalOutput")

    engs = {"sync": nc.sync, "scalar": nc.scalar, "gpsimd": nc.gpsimd}
    store_eng = engs[store_eng]
    b_eng = engs[base_eng]

    with tile.TileContext(nc) as tc:
        with (
            tc.tile_pool(name="const", bufs=1) as const_pool,
            tc.tile_pool(name="xth", bufs=xt_bufs) as xt_pool,
            tc.tile_pool(name="bin", bufs=io_bufs) as b_pool,
            tc.tile_pool(name="oout", bufs=io_bufs) as o_pool,
            tc.tile_pool(name="low", bufs=3) as low_pool,
            tc.tile_pool(name="plow", bufs=plow_bufs, space="PSUM") as plow_pool,
            tc.tile_pool(name="pd2", bufs=pd2_bufs, space="PSUM") as pd2_pool,
            tc.tile_pool(name="pd1", bufs=pd1_bufs, space="PSUM") as pd1_pool,
        ):
            at_sb = const_pool.tile([128, HB, KR], bf16)
            nc.sync.dma_start(at_sb[:], at[:])
            bwt_sb = const_pool.tile([KR, H], bf16)
            nc.sync.dma_start(bwt_sb[:], bwt[:])

            state = {"n_super": 0, "n_single": 0}

            def up_blocks(m, lowT):
                """Up-projection + drain + store for all 4 s-blocks of macro
                m, reading the (already drained) lowT for that macro."""
                for g in range(4):  # 128-row s-blocks within the macro
                    srow = m * 512 + g * S_BLK
                    bt = b_pool.tile([S_BLK, H], bf16, tag="base")
                    b_eng.dma_start(bt[:], base[srow : srow + S_BLK, :])
                    ot = o_pool.tile([S_BLK, H], bf16, tag="out")
                    lg = lowT[:, g * S_BLK : (g + 1) * S_BLK]
                    # 3 paired chunks of 1024 + 1 single of 512 (H = 3584)
                    for c0, width in [(0, 1024), (1024, 1024), (2048, 1024),
                                      (3072, 512)]:
                        if width == 1024:
                            pd = pd2_pool.tile([S_BLK, 1024], f32, tag="pd2")
                            v_path = _spread(state["n_super"] % 48, nv_super, 48)
                            state["n_super"] += 1
                        else:
                            pd = pd1_pool.tile([S_BLK, 512], f32, tag="pd1")
                            v_path = _spread(state["n_single"] % 16, nv_single, 16)
                            state["n_single"] += 1
                        for o in range(0, width, 512):
                            nc.tensor.matmul(
                                pd[:, o : o + 512],
                                lg,
                                bwt_sb[:, c0 + o : c0 + o + 512],
                                start=True,
                                stop=True,
                            )
                        osl = ot[:, c0 : c0 + width]
                        bsl = bt[:, c0 : c0 + width]
                        if v_path:
                            # V: single DVE pass, PSUM operand (1 elem/cyc)
                            nc.vector.tensor_add(osl, pd[:], bsl)
                        else:
                            # C: ACT drains PSUM, DVE adds base in 2x mode
                            nc.scalar.activation(osl, pd[:], Copy)
                            nc.vector.tensor_add(osl, osl, bsl)
                    store_eng.dma_start(out[srow : srow + S_BLK, :], ot[:])

            # pipe=True lags the up-projection one macro behind the
            # down-projection (PE never waits on the fresh lowT drain) —
            # measured slightly worse in the calibrated timeline sim
            # (49.5 vs 49.0 us marginal), so default is the eager schedule.
            prev = None  # (m, lowT) of the previous macro
            for m in range(NMAC * reps):
                m = m % NMAC
                # xT halves: [128 h-partitions, 14 h-blocks, 512 s]
                halves = []
                for hf in range(2):
                    xh = xt_pool.tile([128, HHALF, 512], bf16, tag="xth")
                    nc.sync.dma_start(xh[:], xt[2 * m + hf])
                    halves.append(xh)

                if dma_only:
                    for g in range(4):
                        srow = m * 512 + g * S_BLK
                        bt = b_pool.tile([S_BLK, H], bf16, tag="base")
                        b_eng.dma_start(bt[:], base[srow : srow + S_BLK, :])
                        store_eng.dma_start(out[srow : srow + S_BLK, :], bt[:])
                    continue

                # down-projection: lowT[kr, s] = sum_h At[h, kr] * xT[h, s]
                plow = plow_pool.tile([KR, 512], f32, tag="plow")
                for j in range(HB):
                    nc.tensor.matmul(
                        plow[:],
                        at_sb[:, j, :],
                        halves[j // HHALF][:, j % HHALF, :],
                        start=(j == 0),
                        stop=(j == HB - 1),
                    )
                lowT = low_pool.tile([KR, 512], bf16, tag="lowT")
                nc.scalar.activation(lowT[:], plow[:], Copy)

                if not pipe:
                    up_blocks(m, lowT)
                    continue
                if prev is not None:
                    up_blocks(*prev)
                prev = (m, lowT)
            if prev is not None and not dma_only:
                up_blocks(*prev)

    _split_sync_waits(nc)
    return nc


def make_in_maps(x, base_output, lora_A, lora_B, top_k_weights, top_k_indices):
    """Host-side prep: expert gather, gate/scaling fold, bf16 cast, x h-major
    relayout so every device DMA is large and fully contiguous."""
    x = np.asarray(x, dtype=np.float32)
    base_output = np.asarray(base_output, dtype=np.float32)
    lora_A = np.asarray(lora_A, dtype=np.float32)
    lora_B = np.asarray(lora_B, dtype=np.float32)
    w = np.asarray(top_k_weights, dtype=np.float32)
    idx = np.asarray(top_k_indices)

    A_sel = lora_A[idx]  # [B, K, R, H]
    At = A_sel.reshape(B, KR, H)  # [B, 32, H] (row r = A_cat[r, :])
    # stripe h-major: at[b, p, j, r] = A_cat[b, r, j*128 + p]
    At_dev = np.ascontiguousarray(
        At.reshape(B, KR, HB, 128).transpose(0, 3, 2, 1)
    ).astype(BF16)  # [B, 128, 28, 32]
    B_sel = lora_B[idx]  # [B, K, H, R]
    Bw = B_sel * (w * SCALING)[:, :, None, None]
    Bwt = np.ascontiguousarray(
        Bw.transpose(0, 1, 3, 2).reshape(B, KR, H)
    ).astype(BF16)  # [B, 32, H]

    # x -> xt[half, p, j, s]: h-major tiles, fully contiguous per half
    # xt[b, 2m+hf, p, j, s] = x[b, m*512 + s, hf*1792 + j*128 + p]
    xb = x.astype(BF16)
    xt = np.ascontiguousarray(
        xb.reshape(B, NMAC, 512, 2, HHALF, 128)
        .transpose(0, 1, 3, 5, 4, 2)  # [B, m, hf, p, j, s]
        .reshape(B, 2 * NMAC, 128, HHALF, 512)
    )

    return [
        {
            "xt": xt[b],
            "base": np.ascontiguousarray(base_output[b]).astype(BF16),
            "at": At_dev[b],
            "bwt": Bwt[b],
        }
        for b in range(B)
    ]


def kernel(x, base_output, lora_A, lora_B, top_k_weights, top_k_indices):
    from concourse.bass_utils import run_bass_kernel_spmd

    nc = _CACHE.get("nc")
    if nc is None:
        nc = build_nc()
        _CACHE["nc"] = nc

    in_maps = make_in_maps(
        x, base_output, lora_A, lora_B, top_k_weights, top_k_indices
    )
    res = run_bass_kernel_spmd(nc, in_maps, list(range(N_CORES)))
    return np.stack(
        [res.results[b]["out"].astype(np.float32) for b in range(B)], axis=0
    )
